# revision 3
# baseline (speedup 1.0000x reference)
"""Trainium2 Bass kernel for nn_AutoCorr2D.

Computation (per sample):
  f   = conv3x3(x, w_ext, pad=1) + b_ext            # [CC=128, 64, 64]
  corr[c,i,j,k] = f[c,i,j] * fpad[c, i+u-2, j+v-2]  # 5x5 window products
  out[o,i,j]    = sum_{c,k} w_reg[o,c,k] * corr[c,i,j,k] + b_reg[o]

Sharding: data-parallel over batch B=8 across 8 NeuronCores (one sample per
core); conv weights replicated.

All compute runs in bf16 (inputs host-cast; PSUM accumulation stays f32);
measured end-to-end rel err ~5e-3 vs the f32 reference (tolerance 2e-2).
bf16 buys: 2x DVE rate for the product maps (2x_1p mode needs all-2-byte
packed operands), halved DMA traffic, and PE weight loads short enough to
hide entirely under the previous matmul's stream.

Per-core implementation:
  stage 1: implicit GEMM over (cin_tile, 3x3 tap). Chunks are processed in
           pairs with the tap loop outermost, so each lhsT block serves two
           back-to-back matmuls (half the weight-load pressure); 2x18
           accumulating matmuls per pair into two PSUM banks, bias folded
           into the PSUM->SBUF copy (ScalarE Identity) writing bf16 fpad.
  stage 2: product symmetry: P_{a,b}[y,x] = f[y,x]*f[y+a,x+b] serves both
           tap (a,b) (read at [i,j]) and tap (-a,-b) (read at [i-a,j-b]),
           so only 13 of 25 product maps are computed per 2-chunk group
           (ScalarE Square for (0,0), VectorE bf16 for the rest).  The 25
           regressor matmuls run tap-outer over the group's 2 chunks
           (weight reuse), K=128 channels per tap, into PSUM[64,512],
           bias-copied and DMA'd out.
  Zero-fill memsets (xpad/fpad borders, PE warm-up scratch) run on GpSimd
  so VectorE starts products immediately and the warm-up matmuls (which
  release the HAM clock gate / p-state ramp) issue as early as possible.
"""

import numpy as np
import ml_dtypes

from concourse import bacc, mybir, tile
from concourse.bass_utils import run_bass_kernel_spmd

B, CIN, H, W = 8, 256, 64, 64
CC, COUT = 128, 64
HW = H * W
NCORES = 8

NCHUNK = 8           # pixel chunks per image
CROWS = H // NCHUNK  # rows per chunk (8) -> N = 512 pixels
NPX = CROWS * W      # 512
NGRP = 4             # product-map groups (2 chunks each)
GROWS = 2 * CROWS    # 16

XP = W + 2           # xpad cols (pad=1)
XR = H + 2           # xpad rows
FP = W + 4           # fpad cols (pad=2)
FR = H + 4           # fpad rows
FTAIL = 72           # guard tail so shifted product reads stay in-bounds

# The 13 "upper half" taps; (a,b) also serves tap (-a,-b) via a shifted read.
SYM = [(0, 0), (0, 1), (0, 2),
       (1, -2), (1, -1), (1, 0), (1, 1), (1, 2),
       (2, -2), (2, -1), (2, 0), (2, 1), (2, 2)]

F32 = mybir.dt.float32
BF16 = mybir.dt.bfloat16
U16 = mybir.dt.uint16
AF = mybir.ActivationFunctionType

NWARM = 6


def build_body(nc, tc, x, wext, wreg, bext, breg, out,
               wext_d, wreg_d, bext_d, breg_d):
    with (
        tc.tile_pool(name="const", bufs=1) as constp,
        tc.tile_pool(name="xpadp", bufs=1) as xpadp,
        tc.tile_pool(name="fpadp", bufs=1) as fpadp,
        tc.tile_pool(name="prodp", bufs=2) as prodp,
        tc.tile_pool(name="outp", bufs=2) as outp,
        tc.tile_pool(name="ps1", bufs=3, space="PSUM") as ps1,
        tc.tile_pool(name="ps2", bufs=4, space="PSUM") as ps2,
        tc.tile_pool(name="warmp", bufs=1, space="PSUM") as warmp,
    ):
        # PE warm-up: dummy matmuls on a GpSimd-zeroed bf16 scratch start as
        # soon as the framework preamble ends, releasing the HAM clock gate
        # before real data arrives.
        wsc = constp.tile([128, NPX], BF16, name="wsc")
        nc.gpsimd.memset(wsc.bitcast(U16), 0)
        wpsum = warmp.tile([128, NPX], F32, name="wpsum")
        for i in range(NWARM):
            nc.tensor.matmul(wpsum, wsc[:, :128], wsc,
                             start=(i == 0), stop=(i == NWARM - 1))

        # xpad borders zeroed on GpSimd (VectorE does only products; its
        # queue must not delay stage-1's first chunk).
        xpads = []
        for t in range(2):
            xp = xpadp.tile([128, XR * XP], BF16, name=f"xpad{t}",
                            tag=f"xpad{t}")
            xr = xp.rearrange("p (r c) -> p r c", c=XP)
            xri = xp.bitcast(U16).rearrange("p (r c) -> p r c", c=XP)
            nc.gpsimd.memset(xri[:, 0, :], 0)
            nc.gpsimd.memset(xri[:, XR - 1, :], 0)
            nc.gpsimd.memset(xri[:, 1:XR - 1, 0], 0)
            nc.gpsimd.memset(xri[:, 1:XR - 1, XP - 1], 0)
            xpads.append(xr)

        # Weight DMAs on the Scalar engine's HWDGE queue (parallel to the x
        # stream on Sync); bf16 data lands ready to use - no cast step.
        # Split so the first lhsT blocks (consumed first) arrive earliest.
        WSPLIT = ((0, 3), (3, 9), (9, 18))
        for lo, hi in WSPLIT:
            nc.scalar.dma_start(out=wext[:, lo * 128:hi * 128],
                                in_=wext_d[:, lo * 128:hi * 128])
        nc.scalar.dma_start(out=wreg, in_=wreg_d)

        # x bands DMA straight into the padded xpad interior (bf16, host
        # cast): band i carries exactly the input rows chunk i reads
        # (i*8-1 .. i*8+9, overlapping by 2) - one just-in-time DMA each.
        band_rows = []
        for band in range(NCHUNK):
            ra = max(band * CROWS - 1, 0)
            rb = min(band * CROWS + CROWS + 1, H)
            band_rows.append((ra, rb))
            for t in range(2):
                dst = xpads[t][:, 1 + ra:1 + rb, 1:1 + W]
                src = x[t * 128:(t + 1) * 128, ra * W:rb * W]
                src = src.rearrange("p (r c) -> p r c", c=W)
                nc.sync.dma_start(out=dst, in_=src)
            if band == 0:
                nc.sync.dma_start(out=bext, in_=bext_d)
            elif band == 1:
                nc.sync.dma_start(out=breg, in_=breg_d)

        # ---- padded features (pad=2) + guard tail; borders on GpSimd ----
        fpad = fpadp.tile([128, FR * FP + FTAIL], BF16, name="fpad")
        fr = fpad[:, :FR * FP].rearrange("p (r c) -> p r c", c=FP)
        fi = fpad.bitcast(U16)
        fri = fi[:, :FR * FP].rearrange("p (r c) -> p r c", c=FP)
        nc.gpsimd.memset(fi[:, 0:2 * FP], 0)
        nc.gpsimd.memset(fi[:, (FR - 2) * FP:FR * FP + FTAIL], 0)
        nc.gpsimd.memset(fri[:, 2:FR - 2, 0:2], 0)
        nc.gpsimd.memset(fri[:, 2:FR - 2, FP - 2:FP], 0)

        # ---- stage 1: f = conv3x3(x) + b_ext ----
        # Chunk pairs with the (cin tile, tap) block loop outermost: each
        # lhsT block feeds two back-to-back matmuls into the pair's two
        # PSUM banks.
        for p in range(NCHUNK // 2):
            ca, cb = 2 * p, 2 * p + 1
            pa = ps1.tile([128, NPX], F32, name=f"ps1a{p}", tag="psum1")
            pb = ps1.tile([128, NPX], F32, name=f"ps1b{p}", tag="psum1")
            k = 0
            for t in range(2):
                for du in range(3):
                    for dv in range(3):
                        blk = t * 9 + du * 3 + dv
                        lhsT = wext[:, blk * 128:(blk + 1) * 128]
                        for c, ps in ((ca, pa), (cb, pb)):
                            rhs = xpads[t][:,
                                           c * CROWS + du:
                                           c * CROWS + du + CROWS,
                                           dv:dv + W]
                            nc.tensor.matmul(ps, lhsT, rhs,
                                             start=(k == 0), stop=(k == 17))
                        k += 1
            for c, ps in ((ca, pa), (cb, pb)):
                dst = fr[:, c * CROWS + 2:c * CROWS + 2 + CROWS, 2:2 + W]
                nc.scalar.activation(dst,
                                     ps.rearrange("p (r c) -> p r c", c=W),
                                     AF.Identity, bias=bext, scale=1.0)

        # ---- stage 2: products (2-chunk groups) + regressor GEMM ----
        for g in range(NGRP):
            # product map for tap (a,b): rows [g*16+2-a, g*16+18) of the
            # (-2-origin) padded product grid, full FP-wide rows
            ptiles = []
            for kk, (a, b) in enumerate(SYM):
                nrows = GROWS + a if kk > 0 else GROWS
                base = (g * GROWS + 2 - (a if kk > 0 else 0)) * FP
                pt = prodp.tile([128, nrows * FP], BF16,
                                name=f"prod{kk}", tag=f"prod{kk}", bufs=2)
                in0 = fpad[:, base:base + nrows * FP]
                in1 = fpad[:, base + a * FP + b:
                           base + a * FP + b + nrows * FP]
                if kk == 0:
                    nc.scalar.activation(pt, in0, AF.Square)
                else:
                    nc.vector.tensor_mul(pt, in0, in1)
                ptiles.append(pt)

            # Regressor GEMM: taps outermost, the group's two chunks inner,
            # so consecutive matmuls share the same lhsT.
            psums = []
            for i in (2 * g, 2 * g + 1):
                psums.append((i, ps2.tile([COUT, NPX], F32,
                                          name=f"psum2_{i}", tag="psum2")))
            mm = 0
            for kk, (a, b) in enumerate(SYM):
                pr = ptiles[kk].rearrange("p (r c) -> p r c", c=FP)
                taps = ([(a, b)] if (a, b) == (0, 0)
                        else [(a, b), (-a, -b)])
                for (p, q) in taps:
                    tidx = (p + 2) * 5 + (q + 2)
                    lhsT = wreg[:, tidx * 64:(tidx + 1) * 64]
                    for i, psum2 in psums:
                        p8 = (i % 2) * CROWS
                        if kk == 0:
                            rhs = pr[:, p8:p8 + CROWS, 2:2 + W]
                        elif (p, q) == (a, b):
                            rhs = pr[:, p8 + a:p8 + a + CROWS, 2:2 + W]
                        else:
                            rhs = pr[:, p8:p8 + CROWS, 2 - b:2 - b + W]
                        nc.tensor.matmul(psum2, lhsT, rhs,
                                         start=(mm == 0), stop=(mm == 24))
                    mm += 1

            for i, psum2 in psums:
                outt = outp.tile([COUT, NPX], F32, name="outsb",
                                 tag="outsb")
                nc.scalar.activation(outt, psum2, AF.Identity,
                                     bias=breg, scale=1.0)
                nc.sync.dma_start(out=out[:, i * NPX:(i + 1) * NPX],
                                  in_=outt)


def build_nc():
    nc = bacc.Bacc("TRN2", target_bir_lowering=False, debug=False,
                   num_devices=NCORES)
    x = nc.dram_tensor("x", [CIN, HW], BF16, kind="ExternalInput").ap()
    wext_d = nc.dram_tensor("wext", [128, 18 * 128], BF16,
                            kind="ExternalInput").ap()
    wreg_d = nc.dram_tensor("wreg", [128, 25 * 64], BF16,
                            kind="ExternalInput").ap()
    bext_d = nc.dram_tensor("bext", [128, 1], F32, kind="ExternalInput").ap()
    breg_d = nc.dram_tensor("breg", [64, 1], F32, kind="ExternalInput").ap()
    out = nc.dram_tensor("out", [COUT, HW], F32, kind="ExternalOutput").ap()
    with tile.TileContext(nc) as tc:
        with tc.tile_pool(name="weights", bufs=1) as wp:
            wext = wp.tile([128, 18 * 128], BF16, name="wext_sb")
            wreg = wp.tile([128, 25 * 64], BF16, name="wreg_sb")
            bext = wp.tile([128, 1], F32, name="bext_sb")
            breg = wp.tile([64, 1], F32, name="breg_sb")
            build_body(nc, tc, x, wext, wreg, bext, breg, out,
                       wext_d, wreg_d, bext_d, breg_d)
    nc.compile()
    return nc


def prep_in_maps(x, w_ext, b_ext, w_reg, b_reg):
    x = np.ascontiguousarray(np.asarray(x, dtype=np.float32))
    w_ext = np.asarray(w_ext, dtype=np.float32)
    w_reg = np.asarray(w_reg, dtype=np.float32)
    b_ext = np.asarray(b_ext, dtype=np.float32)
    b_reg = np.asarray(b_reg, dtype=np.float32)

    # lhsT layouts: wext [cin(128-part), (cintile,tap)*cc], wreg [cc, tap*cout]
    w1 = np.transpose(w_ext, (1, 2, 3, 0))          # [CIN, 3, 3, CC]
    wext_p = np.zeros((128, 18, 128), np.float32)
    for t in range(2):
        for du in range(3):
            for dv in range(3):
                wext_p[:, t * 9 + du * 3 + dv, :] = \
                    w1[t * 128:(t + 1) * 128, du, dv, :]
    wext_p = np.ascontiguousarray(wext_p.reshape(128, 18 * 128))
    w2 = np.transpose(w_reg, (1, 2, 3, 0))          # [CC, 5, 5, COUT]
    wreg_p = np.ascontiguousarray(w2.reshape(128, 25 * 64))
    bext_p = np.ascontiguousarray(b_ext.reshape(128, 1))
    breg_p = np.ascontiguousarray(b_reg.reshape(64, 1))

    bf = ml_dtypes.bfloat16
    wext_b = wext_p.astype(bf)
    wreg_b = wreg_p.astype(bf)

    return [{
        "x": np.ascontiguousarray(x[b].reshape(CIN, HW).astype(bf)),
        "wext": wext_b,
        "wreg": wreg_b,
        "bext": bext_p,
        "breg": breg_p,
    } for b in range(B)]


_NC_CACHE = None


def kernel(x, w_ext, b_ext, w_reg, b_reg):
    global _NC_CACHE
    if _NC_CACHE is None:
        _NC_CACHE = build_nc()
    nc = _NC_CACHE
    in_maps = prep_in_maps(x, w_ext, b_ext, w_reg, b_reg)
    res = run_bass_kernel_spmd(nc, in_maps, list(range(NCORES)))
    return np.stack([res.results[b]["out"].reshape(COUT, H, W)
                     for b in range(B)], axis=0)


# revision 4
# speedup vs baseline: 1.0995x; 1.0995x over previous
"""Trainium2 Bass kernel for nn_AutoCorr2D.

Computation (per sample):
  f   = conv3x3(x, w_ext, pad=1) + b_ext            # [CC=128, 64, 64]
  corr[c,i,j,k] = f[c,i,j] * fpad[c, i+u-2, j+v-2]  # 5x5 window products
  out[o,i,j]    = sum_{c,k} w_reg[o,c,k] * corr[c,i,j,k] + b_reg[o]

Sharding: data-parallel over batch B=8 across 8 NeuronCores (one sample per
core); conv weights replicated.

All compute runs in bf16 (inputs host-cast; PSUM accumulation stays f32);
measured end-to-end rel err ~5e-3 vs the f32 reference (tolerance 2e-2).
bf16 buys: 2x DVE rate for the product maps (2x_1p mode needs all-2-byte
packed operands), halved DMA traffic, and PE weight loads short enough to
hide entirely under the previous matmul's stream.

Per-core implementation:
  stage 1: implicit GEMM over (cin_tile, 3x3 tap). Chunks are processed in
           pairs with the tap loop outermost, so each lhsT block serves two
           back-to-back matmuls (half the weight-load pressure); 2x18
           accumulating matmuls per pair into two PSUM banks, bias folded
           into the PSUM->SBUF copy (ScalarE Identity) writing bf16 fpad.
  stage 2: product symmetry: P_{a,b}[y,x] = f[y,x]*f[y+a,x+b] serves both
           tap (a,b) (read at [i,j]) and tap (-a,-b) (read at [i-a,j-b]),
           so only 13 of 25 product maps are computed per 2-chunk group
           (ScalarE Square for (0,0), VectorE bf16 for the rest).  The 25
           regressor matmuls run tap-outer over the group's 2 chunks
           (weight reuse), K=128 channels per tap, into PSUM[64,512],
           bias-copied and DMA'd out.
  Zero-fill memsets (xpad/fpad borders, PE warm-up scratch) run on GpSimd
  so VectorE starts products immediately and the warm-up matmuls (which
  release the HAM clock gate / p-state ramp) issue as early as possible.
"""

import numpy as np
import ml_dtypes

from concourse import bacc, mybir, tile
from concourse.bass_utils import run_bass_kernel_spmd

B, CIN, H, W = 8, 256, 64, 64
CC, COUT = 128, 64
HW = H * W
NCORES = 8

NCHUNK = 8           # pixel chunks per image
CROWS = H // NCHUNK  # rows per chunk (8) -> N = 512 pixels
NPX = CROWS * W      # 512
NGRP = 4             # product-map groups (2 chunks each)
GROWS = 2 * CROWS    # 16

XP = W + 2           # xpad cols (pad=1)
XR = H + 2           # xpad rows
FP = W + 4           # fpad cols (pad=2)
FR = H + 4           # fpad rows
FTAIL = 72           # guard tail so shifted product reads stay in-bounds

# The 13 "upper half" taps; (a,b) also serves tap (-a,-b) via a shifted read.
SYM = [(0, 0), (0, 1), (0, 2),
       (1, -2), (1, -1), (1, 0), (1, 1), (1, 2),
       (2, -2), (2, -1), (2, 0), (2, 1), (2, 2)]

F32 = mybir.dt.float32
BF16 = mybir.dt.bfloat16
U16 = mybir.dt.uint16
AF = mybir.ActivationFunctionType

NWARM = 6


def build_body(nc, tc, x, wext, wreg, bext, breg, out,
               wext_d, wreg_d, bext_d, breg_d):
    with (
        tc.tile_pool(name="const", bufs=1) as constp,
        tc.tile_pool(name="xpadp", bufs=1) as xpadp,
        tc.tile_pool(name="fpadp", bufs=1) as fpadp,
        tc.tile_pool(name="prodp", bufs=2) as prodp,
        tc.tile_pool(name="outp", bufs=2) as outp,
        tc.tile_pool(name="ps1", bufs=3, space="PSUM") as ps1,
        tc.tile_pool(name="ps2", bufs=4, space="PSUM") as ps2,
        tc.tile_pool(name="warmp", bufs=1, space="PSUM") as warmp,
    ):
        # PE warm-up: dummy matmuls on a GpSimd-zeroed bf16 scratch start as
        # soon as the framework preamble ends, releasing the HAM clock gate
        # before real data arrives.
        wsc = constp.tile([128, NPX], BF16, name="wsc")
        nc.gpsimd.memset(wsc.bitcast(U16), 0)
        wpsum = warmp.tile([128, NPX], F32, name="wpsum")
        for i in range(NWARM):
            nc.tensor.matmul(wpsum, wsc[:, :128], wsc,
                             start=(i == 0), stop=(i == NWARM - 1))

        # xpad borders zeroed on GpSimd (VectorE does only products; its
        # queue must not delay stage-1's first chunk).
        xpads = []
        for t in range(2):
            xp = xpadp.tile([128, XR * XP], BF16, name=f"xpad{t}",
                            tag=f"xpad{t}")
            xr = xp.rearrange("p (r c) -> p r c", c=XP)
            xri = xp.bitcast(U16).rearrange("p (r c) -> p r c", c=XP)
            nc.gpsimd.memset(xri[:, 0, :], 0)
            nc.gpsimd.memset(xri[:, XR - 1, :], 0)
            nc.gpsimd.memset(xri[:, 1:XR - 1, 0], 0)
            nc.gpsimd.memset(xri[:, 1:XR - 1, XP - 1], 0)
            xpads.append(xr)

        # Weight DMAs on the Scalar engine's HWDGE queue (parallel to the x
        # stream on Sync); bf16 data lands ready to use - no cast step.
        # Split so the first lhsT blocks (consumed first) arrive earliest.
        WSPLIT = ((0, 3), (3, 9), (9, 18))
        for lo, hi in WSPLIT:
            nc.scalar.dma_start(out=wext[:, lo * 128:hi * 128],
                                in_=wext_d[:, lo * 128:hi * 128])
        nc.scalar.dma_start(out=wreg, in_=wreg_d)

        # x bands DMA straight into the padded xpad interior (bf16, host
        # cast).  Bands are DISJOINT 8-row slabs: overlapping bands would
        # create WAW deps that serialize the DMA queue on completion
        # semaphores (measured ~1.4us/band).  Chunk c reads rows from bands
        # c-1..c+1; the tile framework tracks those read deps directly.
        for band in range(NCHUNK):
            ra, rb = band * CROWS, (band + 1) * CROWS
            for t in range(2):
                dst = xpads[t][:, 1 + ra:1 + rb, 1:1 + W]
                src = x[t * 128:(t + 1) * 128, ra * W:rb * W]
                src = src.rearrange("p (r c) -> p r c", c=W)
                nc.sync.dma_start(out=dst, in_=src)
            if band == 0:
                nc.sync.dma_start(out=bext, in_=bext_d)
            elif band == 1:
                nc.sync.dma_start(out=breg, in_=breg_d)

        # ---- padded features (pad=2) + guard tail; borders on GpSimd ----
        fpad = fpadp.tile([128, FR * FP + FTAIL], BF16, name="fpad")
        fr = fpad[:, :FR * FP].rearrange("p (r c) -> p r c", c=FP)
        fi = fpad.bitcast(U16)
        fri = fi[:, :FR * FP].rearrange("p (r c) -> p r c", c=FP)
        nc.gpsimd.memset(fi[:, 0:2 * FP], 0)
        nc.gpsimd.memset(fi[:, (FR - 2) * FP:FR * FP + FTAIL], 0)
        nc.gpsimd.memset(fri[:, 2:FR - 2, 0:2], 0)
        nc.gpsimd.memset(fri[:, 2:FR - 2, FP - 2:FP], 0)

        # ---- stage 1: f = conv3x3(x) + b_ext ----
        # Chunk pairs with the (cin tile, tap) block loop outermost: each
        # lhsT block feeds two back-to-back matmuls into the pair's two
        # PSUM banks.
        for p in range(NCHUNK // 2):
            ca, cb = 2 * p, 2 * p + 1
            pa = ps1.tile([128, NPX], F32, name=f"ps1a{p}", tag="psum1")
            pb = ps1.tile([128, NPX], F32, name=f"ps1b{p}", tag="psum1")
            k = 0
            for t in range(2):
                for du in range(3):
                    for dv in range(3):
                        blk = t * 9 + du * 3 + dv
                        lhsT = wext[:, blk * 128:(blk + 1) * 128]
                        for c, ps in ((ca, pa), (cb, pb)):
                            rhs = xpads[t][:,
                                           c * CROWS + du:
                                           c * CROWS + du + CROWS,
                                           dv:dv + W]
                            nc.tensor.matmul(ps, lhsT, rhs,
                                             start=(k == 0), stop=(k == 17))
                        k += 1
            for c, ps in ((ca, pa), (cb, pb)):
                dst = fr[:, c * CROWS + 2:c * CROWS + 2 + CROWS, 2:2 + W]
                nc.scalar.activation(dst,
                                     ps.rearrange("p (r c) -> p r c", c=W),
                                     AF.Identity, bias=bext, scale=1.0)

        # ---- stage 2: products (2-chunk groups) + regressor GEMM ----
        for g in range(NGRP):
            # product map for tap (a,b): rows [g*16+2-a, g*16+18) of the
            # (-2-origin) padded product grid, full FP-wide rows
            ptiles = []
            for kk, (a, b) in enumerate(SYM):
                nrows = GROWS + a if kk > 0 else GROWS
                base = (g * GROWS + 2 - (a if kk > 0 else 0)) * FP
                pt = prodp.tile([128, nrows * FP], BF16,
                                name=f"prod{kk}", tag=f"prod{kk}", bufs=2)
                in0 = fpad[:, base:base + nrows * FP]
                in1 = fpad[:, base + a * FP + b:
                           base + a * FP + b + nrows * FP]
                if kk == 0:
                    nc.scalar.activation(pt, in0, AF.Square)
                else:
                    nc.vector.tensor_mul(pt, in0, in1)
                ptiles.append(pt)

            # Regressor GEMM: taps outermost, the group's two chunks inner,
            # so consecutive matmuls share the same lhsT.
            psums = []
            for i in (2 * g, 2 * g + 1):
                psums.append((i, ps2.tile([COUT, NPX], F32,
                                          name=f"psum2_{i}", tag="psum2")))
            mm = 0
            for kk, (a, b) in enumerate(SYM):
                pr = ptiles[kk].rearrange("p (r c) -> p r c", c=FP)
                taps = ([(a, b)] if (a, b) == (0, 0)
                        else [(a, b), (-a, -b)])
                for (p, q) in taps:
                    tidx = (p + 2) * 5 + (q + 2)
                    lhsT = wreg[:, tidx * 64:(tidx + 1) * 64]
                    for i, psum2 in psums:
                        p8 = (i % 2) * CROWS
                        if kk == 0:
                            rhs = pr[:, p8:p8 + CROWS, 2:2 + W]
                        elif (p, q) == (a, b):
                            rhs = pr[:, p8 + a:p8 + a + CROWS, 2:2 + W]
                        else:
                            rhs = pr[:, p8:p8 + CROWS, 2 - b:2 - b + W]
                        nc.tensor.matmul(psum2, lhsT, rhs,
                                         start=(mm == 0), stop=(mm == 24))
                    mm += 1

            for i, psum2 in psums:
                outt = outp.tile([COUT, NPX], F32, name="outsb",
                                 tag="outsb")
                nc.scalar.activation(outt, psum2, AF.Identity,
                                     bias=breg, scale=1.0)
                nc.sync.dma_start(out=out[:, i * NPX:(i + 1) * NPX],
                                  in_=outt)


def build_nc():
    nc = bacc.Bacc("TRN2", target_bir_lowering=False, debug=False,
                   num_devices=NCORES)
    x = nc.dram_tensor("x", [CIN, HW], BF16, kind="ExternalInput").ap()
    wext_d = nc.dram_tensor("wext", [128, 18 * 128], BF16,
                            kind="ExternalInput").ap()
    wreg_d = nc.dram_tensor("wreg", [128, 25 * 64], BF16,
                            kind="ExternalInput").ap()
    bext_d = nc.dram_tensor("bext", [128, 1], F32, kind="ExternalInput").ap()
    breg_d = nc.dram_tensor("breg", [64, 1], F32, kind="ExternalInput").ap()
    out = nc.dram_tensor("out", [COUT, HW], F32, kind="ExternalOutput").ap()
    with tile.TileContext(nc) as tc:
        with tc.tile_pool(name="weights", bufs=1) as wp:
            wext = wp.tile([128, 18 * 128], BF16, name="wext_sb")
            wreg = wp.tile([128, 25 * 64], BF16, name="wreg_sb")
            bext = wp.tile([128, 1], F32, name="bext_sb")
            breg = wp.tile([64, 1], F32, name="breg_sb")
            build_body(nc, tc, x, wext, wreg, bext, breg, out,
                       wext_d, wreg_d, bext_d, breg_d)
    nc.compile()
    return nc


def prep_in_maps(x, w_ext, b_ext, w_reg, b_reg):
    x = np.ascontiguousarray(np.asarray(x, dtype=np.float32))
    w_ext = np.asarray(w_ext, dtype=np.float32)
    w_reg = np.asarray(w_reg, dtype=np.float32)
    b_ext = np.asarray(b_ext, dtype=np.float32)
    b_reg = np.asarray(b_reg, dtype=np.float32)

    # lhsT layouts: wext [cin(128-part), (cintile,tap)*cc], wreg [cc, tap*cout]
    w1 = np.transpose(w_ext, (1, 2, 3, 0))          # [CIN, 3, 3, CC]
    wext_p = np.zeros((128, 18, 128), np.float32)
    for t in range(2):
        for du in range(3):
            for dv in range(3):
                wext_p[:, t * 9 + du * 3 + dv, :] = \
                    w1[t * 128:(t + 1) * 128, du, dv, :]
    wext_p = np.ascontiguousarray(wext_p.reshape(128, 18 * 128))
    w2 = np.transpose(w_reg, (1, 2, 3, 0))          # [CC, 5, 5, COUT]
    wreg_p = np.ascontiguousarray(w2.reshape(128, 25 * 64))
    bext_p = np.ascontiguousarray(b_ext.reshape(128, 1))
    breg_p = np.ascontiguousarray(b_reg.reshape(64, 1))

    bf = ml_dtypes.bfloat16
    wext_b = wext_p.astype(bf)
    wreg_b = wreg_p.astype(bf)

    return [{
        "x": np.ascontiguousarray(x[b].reshape(CIN, HW).astype(bf)),
        "wext": wext_b,
        "wreg": wreg_b,
        "bext": bext_p,
        "breg": breg_p,
    } for b in range(B)]


_NC_CACHE = None


def kernel(x, w_ext, b_ext, w_reg, b_reg):
    global _NC_CACHE
    if _NC_CACHE is None:
        _NC_CACHE = build_nc()
    nc = _NC_CACHE
    in_maps = prep_in_maps(x, w_ext, b_ext, w_reg, b_reg)
    res = run_bass_kernel_spmd(nc, in_maps, list(range(NCORES)))
    return np.stack([res.results[b]["out"].reshape(COUT, H, W)
                     for b in range(B)], axis=0)
